# revision 2
# baseline (speedup 1.0000x reference)
"""Bidirectional chamfer loss on 8 Trainium2 NeuronCores.

Problem: N=16384 render points (128x128x2), M=16384 contour points (16384x2),
output = sum_i min_j ||p_i - q_j|| + sum_j min_i ||p_i - q_j||  (scalar f32).

Strategy (retrieval_knn, v2 — small-window blocks):
  - Host 2D-shards the candidate search: sort both point sets by x, give
    core c the c-th slice of 2048 rows (either direction); within a core,
    sort the slice by y and cut it into 16 blocks of 128. For each block the
    host gathers (data-dependently, via searchsorted) a window of W=256
    candidates from the opposite set restricted to the core's x-range
    +-512 ranks (~16px) and centered on the block's y-range (~12px margin).
    For the fixed harness input this makes the windowed min exact for all
    but ~18 of 32768 rows; an exact host certification bounds the distance
    to every excluded candidate and recomputes failing rows exactly, so the
    kernel is correct for any input.
  - Device (per core): 32 independent 128x256 distance blocks. Each is ONE
    K=10 fp16 matmul into a PSUM bank half (d^2/16 via hi/lo split of
    centered, /4-scaled coordinates: rows xh*xh + xh*xl + xl*xh + (same y)
    + p2h + p2l + q2h + q2l keep fp32-level accuracy at fp16 speed), then a
    DVE reduce_min along the free axis. The reduction — the kernel's
    bottleneck at 1 elem/cycle/lane — only touches 32*256 = 8K columns
    per core instead of the 131K a full-window scheme needs.
  - Host: x16 rescale, sqrt + sums in float64, certification, fallback.
"""

import numpy as np

# ---- hardcoded problem geometry (from the problem spec) ----
N = 16384            # render points (128*128)
M = 16384            # contour points
NCORES = 8
NP_CORE = N // NCORES          # 2048 rows per core per direction
P = 128                        # partitions / block rows
NBLK = NP_CORE // P            # 16 blocks per direction
W = 256                        # candidate window per block
R = 512                        # x-rank margin of the per-core candidate ext
EXT = NP_CORE + 2 * R          # candidate ext size (host-side only)
NB = 2 * NBLK                  # 32 blocks per core (A then B)
GRP = 8                        # blocks per DVE reduce group
NGRP = NB // GRP               # 4 reduce groups
K = 10                         # matmul contraction rows

_COMPILED = {}


def _build_program():
    """SPMD bass program: 32x (K=10 fp16 matmul [128,W] + reduce_min)."""
    import concourse.bass as bass
    from concourse import mybir

    f16 = mybir.dt.float16
    f32 = mybir.dt.float32
    X = mybir.AxisListType.X
    MIN = mybir.AluOpType.min

    nc = bass.Bass("TRN2", target_bir_lowering=False, debug=False,
                   num_devices=NCORES)

    LCOLS = NB * P               # 4096 lhsT columns
    RCOLS = NB * W               # 8192 rhs columns
    inp = nc.dram_tensor("inp", [K, LCOLS + RCOLS], f16,
                         kind="ExternalInput").ap()
    minout = nc.dram_tensor("minout", [P, NB], f32,
                            kind="ExternalOutput").ap()

    with (
        nc.sbuf_tensor([K, LCOLS + RCOLS], f16) as t_inp,
        nc.sbuf_tensor([P, NB], f32) as acc,
        nc.psum_tensor([P, GRP * W], f32) as ps0,
        nc.psum_tensor([P, GRP * W], f32) as ps1,
        nc.semaphore() as dma_sem,
        nc.semaphore() as pe_sem,
        nc.semaphore() as dve_sem,
        nc.Block() as block,
    ):
        @block.sync
        def _(sync):
            sync.dma_start(t_inp[:], inp).then_inc(dma_sem, 16)
            sync.wait_ge(dve_sem, NGRP)
            sync.dma_start(minout, acc[:]).then_inc(dma_sem, 16)

        @block.tensor
        def _(pe):
            pe.wait_ge(dma_sem, 16)
            for g in range(NGRP):
                ps = ps0 if g % 2 == 0 else ps1
                if g >= 2:
                    pe.wait_ge(dve_sem, g - 1)   # slot's previous reduce done
                last = None
                for j in range(GRP):
                    b = g * GRP + j
                    last = nc.tensor.matmul(
                        ps[:, j * W:(j + 1) * W],
                        t_inp[:, b * P:(b + 1) * P],
                        t_inp[:, LCOLS + b * W:LCOLS + (b + 1) * W],
                        start=True, stop=True,
                    )
                last.then_inc(pe_sem, 1)

        @block.vector
        def _(vector):
            for g in range(NGRP):
                ps = ps0 if g % 2 == 0 else ps1
                vector.wait_ge(pe_sem, g + 1)
                nc.vector.tensor_reduce(
                    acc[:, g * GRP:(g + 1) * GRP],
                    ps[:].rearrange("p (j w) -> p j w", w=W),
                    axis=X, op=MIN,
                ).then_inc(dve_sem, 1)

    return nc


def _get_program():
    if "nc" not in _COMPILED:
        _COMPILED["nc"] = _build_program()
    return _COMPILED["nc"]


def _hi_lo(v):
    """fp16 hi/lo split of a float64/float32 array."""
    hi = v.astype(np.float16)
    lo = (v - hi.astype(np.float64)).astype(np.float16)
    return hi, lo


def _block_operands(blk, win):
    """(lhsT [K,128], rhs [K,W]) fp16 for one 128xW distance block.

    PSUM result = d^2/16: coordinates centered on the block+window mean and
    scaled by 1/4, squares split hi/lo so fp16 inputs keep ~1e-4 accuracy.
    """
    c = (blk.mean(axis=0) + win.mean(axis=0)) * 0.5
    bx = (blk[:, 0] - c[0]) * 0.25
    by = (blk[:, 1] - c[1]) * 0.25
    wx = (win[:, 0] - c[0]) * 0.25
    wy = (win[:, 1] - c[1]) * 0.25

    bxh, bxl = _hi_lo(bx)
    byh, byl = _hi_lo(by)
    wxh, wxl = _hi_lo(wx)
    wyh, wyl = _hi_lo(wy)
    p2h, p2l = _hi_lo(bx * bx + by * by)
    q2h, q2l = _hi_lo(wx * wx + wy * wy)
    ones_b = np.ones_like(bxh)
    ones_w = np.ones_like(wxh)

    lhsT = np.stack([bxh, bxh, bxl, byh, byh, byl, p2h, p2l, ones_b, ones_b])
    rhs = np.stack([-2 * wxh, -2 * wxl, -2 * wxh, -2 * wyh, -2 * wyl,
                    -2 * wyh, ones_w, ones_w, q2h, q2l])
    return lhsT.astype(np.float16), rhs.astype(np.float16)


def _prep_side(rows_x, cand_x):
    """Build per-core blocks for one direction.

    rows_x, cand_x: (16384, 2) arrays sorted by x (rows = the side whose
    per-row min this direction computes; cand = the opposite set).

    Returns (lhs_cols, rhs_cols, row_index, cert) where:
      lhs_cols[c]: [K, NBLK*P] fp16, rhs_cols[c]: [K, NBLK*W] fp16
      row_index[c]: (NP_CORE,) indices into rows_x (x-sorted order) in
        device layout order (block-major, then partition)
      cert[c]: dict with exact-bound data for certification.
    """
    Mtot = len(cand_x)
    cx_all = cand_x[:, 0]
    lhs_cols, rhs_cols, row_index, cert = [], [], [], []
    for c in range(NCORES):
        sl = rows_x[c * NP_CORE:(c + 1) * NP_CORE]
        s = int(np.clip(c * NP_CORE - R, 0, Mtot - EXT))
        ext = cand_x[s:s + EXT]
        yo = np.argsort(ext[:, 1], kind="stable")
        ext_y = ext[yo]
        ey = ext_y[:, 1]

        so = np.argsort(sl[:, 1], kind="stable")
        sl_y = sl[so]
        row_index.append(c * NP_CORE + so)

        lhs = np.empty((K, NBLK * P), dtype=np.float16)
        rhs = np.empty((K, NBLK * W), dtype=np.float16)
        wlo = np.empty(NBLK)        # y of last candidate below each window
        whi = np.empty(NBLK)        # y of first candidate above each window
        for b in range(NBLK):
            blk = sl_y[b * P:(b + 1) * P]
            lo = np.searchsorted(ey, blk[:, 1].min())
            hi = np.searchsorted(ey, blk[:, 1].max())
            ws = int(np.clip((lo + hi) // 2 - W // 2, 0, EXT - W))
            win = ext_y[ws:ws + W]
            lhs[:, b * P:(b + 1) * P], rhs[:, b * W:(b + 1) * W] = \
                _block_operands(blk, win)
            wlo[b] = ey[ws - 1] if ws > 0 else -np.inf
            whi[b] = ey[ws + W] if ws + W < EXT else np.inf
        lhs_cols.append(lhs)
        rhs_cols.append(rhs)
        # x-bounds of the ext (for candidates excluded by x-rank)
        xlo = cx_all[s - 1] if s > 0 else -np.inf
        xhi = cx_all[s + EXT] if s + EXT < Mtot else np.inf
        cert.append({"wlo": wlo, "whi": whi, "xlo": xlo, "xhi": xhi})
    return lhs_cols, rhs_cols, row_index, cert


def _certify(rows_sorted, row_index, cert, min2):
    """Exact lower bound on excluded-candidate distance per row; returns
    indices (into device layout order, per core) whose windowed min is not
    certified."""
    bad = []
    for c in range(NCORES):
        idx = row_index[c]
        pts = rows_sorted[idx]          # device-order rows of this core
        px, py = pts[:, 0].astype(np.float64), pts[:, 1].astype(np.float64)
        ct = cert[c]
        b = np.arange(NP_CORE) // P
        dy_lo = np.maximum(py - ct["wlo"][b], 0.0)
        dy_hi = np.maximum(ct["whi"][b] - py, 0.0)
        dx_lo = np.maximum(px - ct["xlo"], 0.0)
        dx_hi = np.maximum(ct["xhi"] - px, 0.0)
        bound2 = np.minimum(np.minimum(dy_lo, dy_hi),
                            np.minimum(dx_lo, dx_hi)) ** 2
        bad.append(np.nonzero(min2[c] > bound2)[0])
    return bad


def kernel(img_render_points: np.ndarray, contour_points: np.ndarray) -> np.ndarray:
    # NOTE: do not enable jax_compilation_cache_dir here — loading this
    # program from the jax persistent cache produces executables that fail
    # with NRT_EXEC_UNIT_UNRECOVERABLE on the axon PJRT path. The NEFF
    # compile itself is cached by the environment's own compile cache.
    from concourse.bass_utils import run_bass_kernel_spmd

    p = np.asarray(img_render_points, dtype=np.float32).reshape(-1, 2)
    q = np.asarray(contour_points, dtype=np.float32)
    assert p.shape == (N, 2) and q.shape == (M, 2)

    po = np.argsort(p[:, 0], kind="stable")
    qo = np.argsort(q[:, 0], kind="stable")
    ps = p[po]
    qs = q[qo]

    a_lhs, a_rhs, a_idx, a_cert = _prep_side(ps, qs)   # rows = p, cand = q
    b_lhs, b_rhs, b_idx, b_cert = _prep_side(qs, ps)   # rows = q, cand = p

    in_maps = []
    for c in range(NCORES):
        inp = np.concatenate([a_lhs[c], b_lhs[c], a_rhs[c], b_rhs[c]],
                             axis=1)
        in_maps.append({"inp": np.ascontiguousarray(inp)})

    nc = _get_program()
    res = run_bass_kernel_spmd(nc, in_maps, list(range(NCORES)))
    results = res.results

    # ---- unpack device mins (d^2, per core, device layout order) ----
    amin2 = []   # [NCORES][NP_CORE] row mins for p-side
    bmin2 = []
    for c in range(NCORES):
        mo = np.asarray(results[c]["minout"], dtype=np.float64) * 16.0
        # minout[r, b] -> device row b*P + r
        amin2.append(mo[:, :NBLK].T.reshape(-1))
        bmin2.append(mo[:, NBLK:].T.reshape(-1))

    # ---- exact certification + fallback ----
    bad_a = _certify(ps, a_idx, a_cert, amin2)
    bad_b = _certify(qs, b_idx, b_cert, bmin2)

    qd = qs.astype(np.float64)
    pd = ps.astype(np.float64)
    for c in range(NCORES):
        if bad_a[c].size:
            rows = ps[a_idx[c][bad_a[c]]].astype(np.float64)
            d2 = ((rows[:, None, :] - qd[None, :, :]) ** 2).sum(-1)
            amin2[c][bad_a[c]] = d2.min(axis=1)
        if bad_b[c].size:
            rows = qs[b_idx[c][bad_b[c]]].astype(np.float64)
            d2 = ((rows[:, None, :] - pd[None, :, :]) ** 2).sum(-1)
            bmin2[c][bad_b[c]] = d2.min(axis=1)

    total = 0.0
    for c in range(NCORES):
        total += np.sqrt(np.maximum(amin2[c], 0.0)).sum()
        total += np.sqrt(np.maximum(bmin2[c], 0.0)).sum()
    return np.float32(total)
